# revision 1
# baseline (speedup 1.0000x reference)
"""Trainium2 Bass kernel for nn_MoEDetector (moe_routing).

Strategy: data-parallel over batch B=8 -> one batch per NeuronCore.
Per-core program (all activations SBUF-resident, no DRAM spills):
  - router logits in fp32 (argmax-selection safe), group softmax ratios
  - GCN1 -> agg -> GCN2 -> agg -> residual+LayerNorm, matmuls in bf16
  - 7 expert matmuls (3 syn on LN output, 1 len + 3 sem on hs), exact
    gelu, per-token top-1 selection folded into per-token coefficients
  - cls projection
Host-side simplifications (exact):
  - the active len expert (short vs long) is fully determined by
    seq_lengths[b] (router masking forces the argmax), so each core
    gets only the active len weight and a 7-column router matrix
  - LN gain/bias folded into the syn expert weights
  - zero biases (the spec fills) are skipped; nonzero biases are
    supported via an extra K=1 rank-1 matmul accumulation step
"""

import numpy as np
import ml_dtypes
from contextlib import ExitStack

B, S, H = 8, 1024, 1536
THRESHOLD = 128
P = 128
ST = S // P          # 8 s-tiles
KT = H // P          # 12 h contraction tiles
TT = S // P          # 8 t-tiles for adjacency contraction
NCH = 512            # matmul moving free-dim chunk
NN = H // NCH        # 3 chunks of the H output dim
EPS = 1e-5

_BF16 = ml_dtypes.bfloat16

_prog_cache = {}


def _build_program(cfg, debug_taps=False):
    """cfg = (router_bias_nz, syn_bias_nz, len_bias_nz, sem_bias_nz, cls_bias_nz)"""
    import concourse.bass as bass
    import concourse.tile as tile
    from concourse import bacc, masks, mybir

    rb_nz, synb_nz, lenb_nz, semb_nz, clsb_nz = cfg
    f32 = mybir.dt.float32
    bf16 = mybir.dt.bfloat16
    AF = mybir.ActivationFunctionType
    ALU = mybir.AluOpType
    AX = mybir.AxisListType
    ts = bass.ts

    nc = bacc.Bacc("TRN2", target_bir_lowering=False, debug=False)

    # ---- DRAM I/O ----
    hs_d = nc.dram_tensor("hs", [S, H], f32, kind="ExternalInput").ap()
    adj_d = nc.dram_tensor("adj", [S, S], f32, kind="ExternalInput").ap()
    rw_d = nc.dram_tensor("rw", [H, 7], f32, kind="ExternalInput").ap()
    wg1_d = nc.dram_tensor("wg1", [H, H], bf16, kind="ExternalInput").ap()
    wg2_d = nc.dram_tensor("wg2", [H, H], bf16, kind="ExternalInput").ap()
    wsyn_d = nc.dram_tensor("wsyn", [3, H, H], bf16, kind="ExternalInput").ap()
    wlen_d = nc.dram_tensor("wlen", [H, H], bf16, kind="ExternalInput").ap()
    wsem_d = nc.dram_tensor("wsem", [3, H, H], bf16, kind="ExternalInput").ap()
    wcls_d = nc.dram_tensor("wcls", [H, 2], bf16, kind="ExternalInput").ap()
    br_d = nc.dram_tensor("br", [1, 7], f32, kind="ExternalInput").ap() if rb_nz else None
    bsyn_d = nc.dram_tensor("bsyn", [3, H], f32, kind="ExternalInput").ap() if synb_nz else None
    blen_d = nc.dram_tensor("blen", [1, H], f32, kind="ExternalInput").ap() if lenb_nz else None
    bsem_d = nc.dram_tensor("bsem", [3, H], f32, kind="ExternalInput").ap() if semb_nz else None
    bcls_d = nc.dram_tensor("bcls", [1, 2], f32, kind="ExternalInput").ap() if clsb_nz else None
    out_d = nc.dram_tensor("out", [S, 2], f32, kind="ExternalOutput").ap()
    taps = {}
    if debug_taps:
        for nm, shape, dt in [
            ("d_logit", [S, 7], f32), ("d_coef", [S, 7], f32),
            ("d_sup1", [S, H], bf16), ("d_x1T", [H, S], bf16),
            ("d_shared", [S, H], f32), ("d_fused", [S, H], f32),
            ("d_adjT", [S, S], bf16), ("d_hsT", [H, S], bf16),
        ]:
            taps[nm] = nc.dram_tensor(nm, shape, dt, kind="ExternalOutput").ap()

    hs_r = hs_d.rearrange("(a p) h -> p a h", p=P)
    adj_r = adj_d.rearrange("(a p) t -> p a t", p=P)
    rw_r = rw_d.rearrange("(k p) e -> p k e", p=P)
    wcls_r = wcls_d.rearrange("(k p) c -> p k c", p=P)
    out_r = out_d.rearrange("(a p) c -> p a c", p=P)

    with tile.TileContext(nc) as tc, ExitStack() as ctx:
        # ---- pools ----
        const = ctx.enter_context(tc.tile_pool(name="const", bufs=1))
        hspool = ctx.enter_context(tc.tile_pool(name="hspool", bufs=1))
        hstpool = ctx.enter_context(tc.tile_pool(name="hstpool", bufs=1))
        bigT = ctx.enter_context(tc.tile_pool(name="bigT", bufs=1))
        adjpool = ctx.enter_context(tc.tile_pool(name="adjpool", bufs=1))
        suppool = ctx.enter_context(tc.tile_pool(name="suppool", bufs=1))
        wpool = ctx.enter_context(tc.tile_pool(name="wpool", bufs=13))
        small = ctx.enter_context(tc.tile_pool(name="small", bufs=2))
        trans = ctx.enter_context(tc.tile_pool(name="trans", bufs=2))
        rowf32 = ctx.enter_context(tc.tile_pool(name="rowf32", bufs=2))
        acc = ctx.enter_context(tc.tile_pool(name="acc", bufs=4, space="PSUM"))
        tp = ctx.enter_context(tc.tile_pool(name="tp", bufs=3, space="PSUM"))
        spsum = ctx.enter_context(tc.tile_pool(name="spsum", bufs=1, space="PSUM"))

        # ---- constants ----
        id_f32 = const.tile([P, P], f32, tag="idf")
        masks.make_identity(nc, id_f32[:])
        id_bf = const.tile([P, P], bf16, tag="idb")
        masks.make_identity(nc, id_bf[:])
        rw_sb = const.tile([P, KT, 7], f32, tag="rw")
        nc.gpsimd.dma_start(rw_sb[:], rw_r)
        wcls_sb = const.tile([P, KT, 2], bf16, tag="wcls")
        nc.gpsimd.dma_start(wcls_sb[:], wcls_r)
        eps_t = const.tile([P, 1], f32, tag="eps")
        nc.vector.memset(eps_t[:], EPS)
        ones_row = None
        if any(x is not None for x in (br_d, bsyn_d, blen_d, bsem_d, bcls_d)):
            ones_row = const.tile([1, P], f32, tag="ones")
            nc.vector.memset(ones_row[:], 1.0)

        def bias_row(dram_ap, n, tag):
            t = const.tile([1, n], f32, tag=tag)
            nc.gpsimd.dma_start(t[:], dram_ap)
            return t

        br_sb = bias_row(br_d, 7, "br") if br_d is not None else None
        bsyn_sb = ([bias_row(bsyn_d[e : e + 1, :], H, f"bsyn{e}") for e in range(3)]
                   if bsyn_d is not None else None)
        blen_sb = bias_row(blen_d, H, "blen") if blen_d is not None else None
        bsem_sb = ([bias_row(bsem_d[e : e + 1, :], H, f"bsem{e}") for e in range(3)]
                   if bsem_d is not None else None)

        # ---- adjacency: degree-normalize rows, transpose to [t, s] bf16 ----
        # adj_bf shares the "sup" slot (it is dead before sup1 is written)
        adj_bf = suppool.tile([P, ST, S], bf16, tag="sup")
        adjT = adjpool.tile([P, TT, S], bf16, tag="adjT")
        for a in range(ST):
            araw = rowf32.tile([P, S], f32, tag="rowf32")
            nc.sync.dma_start(araw[:], adj_r[:, a, :])
            deg = small.tile([P, 1], f32, tag=f"deg{a}")
            nc.vector.tensor_reduce(deg[:], araw[:], axis=AX.X, op=ALU.add)
            nc.vector.tensor_scalar_max(deg[:], deg[:], 1e-9)
            nc.vector.reciprocal(deg[:], deg[:])
            nc.vector.tensor_scalar_mul(adj_bf[:, a, :], araw[:], deg[:])
        for a in range(ST):
            for t in range(TT):
                pt = tp.tile([P, P], bf16, tag="tp")
                nc.tensor.transpose(pt[:], adj_bf[:, a, ts(t, P)], id_bf[:])
                nc.any.tensor_copy(adjT[:, t, ts(a, P)], pt[:])

        # ---- hs load; transpose to hsT (fp32 pass feeds the router) ----
        hs_all = hspool.tile([P, ST, H], f32, tag="hs")
        for a in range(ST):
            nc.sync.dma_start(hs_all[:, a, :], hs_r[:, a, :])
        # Router: single-shot matmul per (k, m) into PSUM, accumulate over k
        # on the vector engine in SBUF. (start=True clears has_written for
        # the WHOLE bank, so interleaved per-m accumulation groups sharing
        # one bank corrupt each other — values survive, bits don't.)
        hsT = hstpool.tile([P, KT, S], bf16, tag="hsT")
        logit = small.tile([P, ST, 7], f32, tag="logit")
        nc.vector.memset(logit[:], 0.0)
        for k in range(KT):
            hTf = rowf32.tile([P, S], f32, tag="rowf32")
            for a in range(ST):
                pt = tp.tile([P, P], f32, tag="tp")
                nc.tensor.transpose(pt[:], hs_all[:, a, ts(k, P)], id_f32[:])
                nc.any.tensor_copy(hTf[:, ts(a, P)], pt[:])
                nc.any.tensor_copy(hsT[:, k, ts(a, P)], pt[:])
            rlog = spsum.tile([P, ST, 7], f32, tag="sp")
            for m in range(ST):
                nc.tensor.matmul(rlog[:, m, :], hTf[:, ts(m, P)], rw_sb[:, k, :],
                                 start=True, stop=True)
            nc.vector.tensor_add(logit[:], logit[:], rlog[:])
        if br_sb is not None:
            rlog = spsum.tile([P, ST, 7], f32, tag="sp")
            for m in range(ST):
                nc.tensor.matmul(rlog[:, m, :], ones_row[:], br_sb[:],
                                 start=True, stop=True)
            nc.vector.tensor_add(logit[:], logit[:], rlog[:])

        # ---- router math: group softmax ratios + top-1 coefficients ----
        # logits are O(1): exp() without max-subtraction is safe, and softmax
        # ratios are shift-invariant so this matches the reference exactly.
        e_sb = small.tile([P, ST, 7], f32, tag="esb")
        nc.scalar.activation(e_sb[:], logit[:], AF.Exp)
        syn_e = small.tile([P, ST], f32, tag="syn_e")
        nc.vector.tensor_reduce(syn_e[:], e_sb[:, :, 0:3], axis=AX.X, op=ALU.max)
        sem_e = small.tile([P, ST], f32, tag="sem_e")
        nc.vector.tensor_reduce(sem_e[:], e_sb[:, :, 4:7], axis=AX.X, op=ALU.max)
        rden = small.tile([P, ST], f32, tag="rden")
        nc.vector.tensor_add(rden[:], syn_e[:], sem_e[:])
        nc.vector.tensor_add(rden[:], rden[:], e_sb[:, :, 3])
        nc.vector.reciprocal(rden[:], rden[:])

        csyn = small.tile([P, ST, 3], f32, tag="csyn")
        csem = small.tile([P, ST, 3], f32, tag="csem")
        clen = small.tile([P, ST], f32, tag="clen")
        nc.vector.tensor_mul(clen[:], e_sb[:, :, 3], rden[:])

        def group_coefs(cout, base, w_e):
            """cout[:,:,e] = rden * w_e * mask_e; first-max argmax over logit
            columns base..base+2 (matches jnp.argmax tie-breaking)."""
            l0, l1, l2 = (logit[:, :, base + i] for i in range(3))
            s0 = small.tile([P, ST], f32, tag="s0")
            ge02 = small.tile([P, ST], f32, tag="ge02")
            nc.vector.tensor_tensor(out=s0[:], in0=l0, in1=l1, op=ALU.is_ge)
            nc.vector.tensor_tensor(out=ge02[:], in0=l0, in1=l2, op=ALU.is_ge)
            nc.vector.tensor_mul(s0[:], s0[:], ge02[:])
            s1 = small.tile([P, ST], f32, tag="s1")
            ge12 = small.tile([P, ST], f32, tag="ge12")
            nc.vector.tensor_tensor(out=ge12[:], in0=l1, in1=l2, op=ALU.is_ge)
            nc.vector.tensor_mul(s1[:], s0[:], ge12[:])
            nc.vector.tensor_tensor(out=s1[:], in0=ge12[:], in1=s1[:], op=ALU.subtract)
            s2 = small.tile([P, ST], f32, tag="s2")
            nc.vector.tensor_add(s2[:], s0[:], s1[:])
            nc.vector.tensor_scalar(out=s2[:], in0=s2[:], scalar1=-1.0, scalar2=1.0,
                                    op0=ALU.mult, op1=ALU.add)
            for e, sm in enumerate((s0, s1, s2)):
                nc.vector.tensor_mul(cout[:, :, e], sm[:], w_e)
                nc.vector.tensor_mul(cout[:, :, e], cout[:, :, e], rden[:])

        group_coefs(csyn, 0, syn_e[:])
        group_coefs(csem, 4, sem_e[:])

        if debug_taps:
            lr = taps["d_logit"].rearrange("(a p) e -> p a e", p=P)
            nc.gpsimd.dma_start(lr, logit[:])
            cr = taps["d_coef"].rearrange("(a p) e -> p a e", p=P)
            nc.gpsimd.dma_start(cr[:, :, 0:3], csyn[:])
            nc.gpsimd.dma_start(cr[:, :, 3:4], clen[:])
            nc.gpsimd.dma_start(cr[:, :, 4:7], csem[:])
            nc.gpsimd.dma_start(
                taps["d_hsT"].rearrange("(k p) s -> p k s", p=P), hsT[:])
            nc.gpsimd.dma_start(
                taps["d_adjT"].rearrange("(t p) s -> p t s", p=P), adjT[:])

        # ---- helpers for the dense [S,H] x [H,H] matmuls ----
        def load_wtiles(wdram):
            tiles = []
            for k in range(KT):
                wt = wpool.tile([P, H], bf16, tag="w")
                nc.sync.dma_start(wt[:], wdram[ts(k, P), :])
                tiles.append(wt)
            return tiles

        def weight_mm(lhsT_t, wtiles, evict, bias_sb=None):
            """evict(m, n, psum) with psum = (lhsT.T @ W + bias)[m-tile, n-chunk]"""
            for m in range(ST):
                for n in range(NN):
                    ps = acc.tile([P, NCH], f32, tag="acc")
                    for k in range(KT):
                        last = (k == KT - 1) and (bias_sb is None)
                        nc.tensor.matmul(ps[:], lhsT_t[:, k, ts(m, P)],
                                         wtiles[k][:, ts(n, NCH)],
                                         start=(k == 0), stop=last)
                    if bias_sb is not None:
                        nc.tensor.matmul(ps[:], ones_row[:],
                                         bias_sb[:, ts(n, NCH)],
                                         start=False, stop=True)
                    evict(m, n, ps)

        def transpose_into(dstT, src_of_k, m, ident):
            for k in range(KT):
                pt = tp.tile([P, P], ident.dtype, tag="tp")
                nc.tensor.transpose(pt[:], src_of_k(k), ident[:])
                nc.any.tensor_copy(dstT[:, k, ts(m, P)], pt[:])

        # ---- GCN layer 1 ----
        w_g1 = load_wtiles(wg1_d)
        sup1 = suppool.tile([P, TT, H], bf16, tag="sup")
        weight_mm(hsT, w_g1,
                  lambda m, n, ps: nc.vector.tensor_copy(sup1[:, m, ts(n, NCH)], ps[:]))
        if debug_taps:
            nc.gpsimd.dma_start(
                taps["d_sup1"].rearrange("(a p) h -> p a h", p=P), sup1[:])

        x1T = bigT.tile([P, KT, S], bf16, tag="bigT")
        for m in range(ST):
            x1row = trans.tile([P, H], bf16, tag="x1row")
            for n in range(NN):
                ps = acc.tile([P, NCH], f32, tag="acc")
                for t in range(TT):
                    nc.tensor.matmul(ps[:], adjT[:, t, ts(m, P)],
                                     sup1[:, t, ts(n, NCH)],
                                     start=(t == 0), stop=(t == TT - 1))
                nc.scalar.activation(x1row[:, ts(n, NCH)], ps[:], AF.Relu)
            transpose_into(x1T, lambda k: x1row[:, ts(k, P)], m, id_bf)

        if debug_taps:
            nc.gpsimd.dma_start(
                taps["d_x1T"].rearrange("(k p) s -> p k s", p=P), x1T[:])

        # ---- GCN layer 2 ----
        w_g2 = load_wtiles(wg2_d)
        sup2 = suppool.tile([P, TT, H], bf16, tag="sup")
        weight_mm(x1T, w_g2,
                  lambda m, n, ps: nc.vector.tensor_copy(sup2[:, m, ts(n, NCH)], ps[:]))

        # ---- agg2 + residual + LayerNorm (affine folded into syn weights) ----
        sharedT = bigT.tile([P, KT, S], bf16, tag="bigT")
        for m in range(ST):
            x2row = trans.tile([P, H], f32, tag="rowbig")
            for n in range(NN):
                ps = acc.tile([P, NCH], f32, tag="acc")
                for t in range(TT):
                    nc.tensor.matmul(ps[:], adjT[:, t, ts(m, P)],
                                     sup2[:, t, ts(n, NCH)],
                                     start=(t == 0), stop=(t == TT - 1))
                nc.scalar.activation(x2row[:, ts(n, NCH)], ps[:], AF.Relu)
            nc.vector.tensor_add(hs_all[:, m, :], hs_all[:, m, :], x2row[:])
            stats = small.tile([P, NN, 6], f32, tag="stats")
            for c in range(NN):
                nc.vector.bn_stats(stats[:, c, :], hs_all[:, m, ts(c, NCH)])
            mv = small.tile([P, 2], f32, tag="mv")
            nc.vector.bn_aggr(mv[:], stats[:])
            rstd = small.tile([P, 1], f32, tag="rstd")
            nc.scalar.activation(rstd[:], mv[:, 1:2], AF.Sqrt, bias=eps_t[:])
            nc.vector.reciprocal(rstd[:], rstd[:])
            nc.vector.tensor_scalar(out=hs_all[:, m, :], in0=hs_all[:, m, :],
                                    scalar1=mv[:, 0:1], scalar2=rstd[:],
                                    op0=ALU.subtract, op1=ALU.mult)
            transpose_into(sharedT, lambda k: hs_all[:, m, ts(k, P)], m, id_f32)

        if debug_taps:
            nc.gpsimd.dma_start(
                taps["d_shared"].rearrange("(a p) h -> p a h", p=P), hs_all[:])

        # ---- experts: gelu + weighted top-1 accumulation into hs_all ----
        experts = [(wsyn_d[e], sharedT, csyn[:, :, e],
                    bsyn_sb[e] if bsyn_sb else None) for e in range(3)]
        experts.append((wlen_d, hsT, clen[:, :], blen_sb))
        experts += [(wsem_d[e], hsT, csem[:, :, e],
                     bsem_sb[e] if bsem_sb else None) for e in range(3)]

        for ei, (wdram, lhsT_t, coef, bias_sb) in enumerate(experts):
            wt = load_wtiles(wdram)

            def evict(m, n, ps, ei=ei, coef=coef):
                g = trans.tile([P, NCH], f32, tag="rowbig")
                nc.scalar.activation(g[:], ps[:], AF.Gelu)
                dst = hs_all[:, m, ts(n, NCH)]
                if ei == 0:
                    nc.vector.tensor_scalar_mul(dst, g[:], coef[:, m : m + 1])
                else:
                    nc.vector.scalar_tensor_tensor(
                        out=dst, in0=g[:], scalar=coef[:, m : m + 1], in1=dst,
                        op0=ALU.mult, op1=ALU.add)

            weight_mm(lhsT_t, wt, evict, bias_sb=bias_sb)

        if debug_taps:
            nc.gpsimd.dma_start(
                taps["d_fused"].rearrange("(a p) h -> p a h", p=P), hs_all[:])

        # ---- fusedT + cls projection ----
        bcls_sb = bias_row(bcls_d, 2, "bcls") if bcls_d is not None else None
        fusedT = bigT.tile([P, KT, S], bf16, tag="bigT")
        cps = spsum.tile([P, ST, 2], f32, tag="sp")
        out_sb = small.tile([P, ST, 2], f32, tag="outsb")
        for m in range(ST):
            transpose_into(fusedT, lambda k: hs_all[:, m, ts(k, P)], m, id_f32)
            for k in range(KT):
                last = (k == KT - 1) and (bcls_sb is None)
                nc.tensor.matmul(cps[:, m, :], fusedT[:, k, ts(m, P)],
                                 wcls_sb[:, k, :], start=(k == 0), stop=last)
            if bcls_sb is not None:
                nc.tensor.matmul(cps[:, m, :], ones_row[:], bcls_sb[:],
                                 start=False, stop=True)
            nc.any.tensor_copy(out_sb[:, m, :], cps[:, m, :])
        nc.gpsimd.dma_start(out_r, out_sb[:])

    nc.compile()
    return nc


def _get_program(cfg):
    if cfg not in _prog_cache:
        _prog_cache[cfg] = _build_program(cfg)
    return _prog_cache[cfg]


def kernel(**inputs):
    from concourse import bass_utils

    hs = np.asarray(inputs["hidden_states"], dtype=np.float32)
    adj = np.asarray(inputs["adj_matrix"], dtype=np.float32)
    seq_lengths = np.asarray(inputs["seq_lengths"])
    router_w = np.asarray(inputs["router_w"], dtype=np.float32)
    router_b = np.asarray(inputs["router_b"], dtype=np.float32)
    gcn1_w = np.asarray(inputs["gcn1_w"], dtype=np.float32)
    gcn2_w = np.asarray(inputs["gcn2_w"], dtype=np.float32)
    ln_g = np.asarray(inputs["ln_g"], dtype=np.float32)
    ln_b = np.asarray(inputs["ln_b"], dtype=np.float32)
    syn_w = np.asarray(inputs["syn_w"], dtype=np.float32)
    syn_b = np.asarray(inputs["syn_b"], dtype=np.float32)
    len_short_w = np.asarray(inputs["len_short_w"], dtype=np.float32)
    len_short_b = np.asarray(inputs["len_short_b"], dtype=np.float32)
    len_long_w = np.asarray(inputs["len_long_w"], dtype=np.float32)
    len_long_b = np.asarray(inputs["len_long_b"], dtype=np.float32)
    sem_w = np.asarray(inputs["sem_w"], dtype=np.float32)
    sem_b = np.asarray(inputs["sem_b"], dtype=np.float32)
    cls_w = np.asarray(inputs["cls_w"], dtype=np.float32)
    cls_b = np.asarray(inputs["cls_b"], dtype=np.float32)

    # fold LN affine into syn expert weights: (x*g + b) @ W = x @ (g[:,None]*W) + b@W
    syn_w_f = (ln_g[None, :, None] * syn_w).astype(np.float32)
    syn_b_f = (syn_b + np.einsum("h,ehd->ed", ln_b, syn_w)).astype(np.float32)

    is_short = seq_lengths <= THRESHOLD

    cfg = (
        bool(np.any(router_b != 0)),
        bool(np.any(syn_b_f != 0)),
        bool(np.any(len_short_b != 0) or np.any(len_long_b != 0)),
        bool(np.any(sem_b != 0)),
        bool(np.any(cls_b != 0)),
    )
    nc = _get_program(cfg)

    wg1 = gcn1_w.astype(_BF16)
    wg2 = gcn2_w.astype(_BF16)
    wsyn = syn_w_f.astype(_BF16)
    wlen_s = len_short_w.astype(_BF16)
    wlen_l = len_long_w.astype(_BF16)
    wsem = sem_w.astype(_BF16)
    wcls = cls_w.astype(_BF16)

    in_maps = []
    for b in range(B):
        lencol = 3 if is_short[b] else 4
        rw7 = np.ascontiguousarray(np.concatenate(
            [router_w[:, 0:3], router_w[:, lencol : lencol + 1], router_w[:, 5:8]],
            axis=1, dtype=np.float32))
        m = {
            "hs": np.ascontiguousarray(hs[b]),
            "adj": np.ascontiguousarray(adj[b]),
            "rw": rw7,
            "wg1": wg1, "wg2": wg2, "wsyn": wsyn,
            "wlen": wlen_s if is_short[b] else wlen_l,
            "wsem": wsem, "wcls": wcls,
        }
        if cfg[0]:
            br7 = np.concatenate(
                [router_b[0:3], router_b[lencol : lencol + 1], router_b[5:8]])
            m["br"] = br7.reshape(1, 7).astype(np.float32)
        if cfg[1]:
            m["bsyn"] = syn_b_f
        if cfg[2]:
            m["blen"] = (len_short_b if is_short[b]
                         else len_long_b).reshape(1, H).astype(np.float32)
        if cfg[3]:
            m["bsem"] = sem_b.astype(np.float32)
        if cfg[4]:
            m["bcls"] = cls_b.reshape(1, 2).astype(np.float32)
        in_maps.append(m)

    try:
        res = bass_utils.run_bass_kernel_spmd(nc, in_maps, core_ids=list(range(B)))
    except Exception:
        # transient device wedge (NRT_EXEC_UNIT_UNRECOVERABLE) clears on retry
        res = bass_utils.run_bass_kernel_spmd(nc, in_maps, core_ids=list(range(B)))
    globals()["_last_results"] = res
    out = np.stack([res.results[b]["out"] for b in range(B)]).astype(np.float32)
    return out



# revision 9
# speedup vs baseline: 1.5829x; 1.5829x over previous
"""Trainium2 Bass kernel for nn_MoEDetector (moe_routing).

Strategy: data-parallel over batch B=8 -> one batch per NeuronCore, plus
top-1 expert bucketing so the syn/sem groups run ~3/8 of the dense work.

Host side (cheap, exact):
  - router logits/probs/argmax + group coefficients in fp32 numpy
    (top-2 logit gaps are ~1e-4 while fp32 sum-order noise is ~1e-6, so
    the argmax always matches the jax reference)
  - tokens sorted by syn expert (perm applied to hs, adj rows+cols);
    second sort by sem expert gives hs_sem
  - per-expert compile-time column WINDOWS [re_e, re_e+cap_e*128) that
    cover the bucket on every core (offsets differ per core; the window
    union is compile-time, per-core masking via zeroed coefficients)
  - adjacency degree-normalized + transposed, hs transposed, both bf16
  - final per-group cls outputs are unpermuted and summed on host

Device program (shared by all 8 cores; per-core tensor CONTENT differs):
  - GCN1 -> agg1(relu) -> GCN2 -> agg2(relu) -> +hs residual -> LayerNorm
    (affine folded into syn weights), matmuls bf16, accumulation fp32
  - experts run TRANSPOSED: out_T[d,tok] = W^T @ x_T, so the gelu output
    lands pre-transposed for the cls projection and the per-token
    coefficient factors out of the d-contraction -> applied after cls as
    a per-partition scalar on the [slots,2] result
  - groups: syn (3 windows on sharedT), len (all tokens on hsT),
    sem (3 windows on hs_semT); each -> fusedT bf16 -> cls -> out rows
"""

import numpy as np
import ml_dtypes
from contextlib import ExitStack

B, S, H = 8, 1024, 1536
THRESHOLD = 128
NEG = -1e9
P = 128
ST = S // P          # 8 token tiles
KT = H // P          # 12 h tiles
TT = S // P          # 8 t tiles
NCH = 512            # matmul moving free-dim chunk
NN = H // NCH        # 3 chunks of H
EPS = 1e-5
SPAD = 384           # pad tail so expert windows may overrun S

_BF16 = ml_dtypes.bfloat16

_prog_cache = {}


# ---------------------------------------------------------------- host math
def _route_host(hs, rw, rb, seq_lengths):
    """fp32 numpy replication of the reference router."""
    logits = (hs.reshape(-1, H).astype(np.float32) @ rw).reshape(B, S, 8) + rb
    is_short = (np.asarray(seq_lengths) <= THRESHOLD)
    lg = logits.copy()
    lg[..., 3] = np.where(is_short[:, None], logits[..., 3], NEG)
    lg[..., 4] = np.where(is_short[:, None], NEG, logits[..., 4])
    m = lg.max(-1, keepdims=True)
    e = np.exp((lg - m).astype(np.float32))
    probs = (e / e.sum(-1, keepdims=True)).astype(np.float32)
    syn_p = probs[..., 0:3].max(-1)
    syn_i = probs[..., 0:3].argmax(-1)
    len_p = probs[..., 3:5].max(-1)
    sem_p = probs[..., 5:8].max(-1)
    sem_i = probs[..., 5:8].argmax(-1)
    den = syn_p + len_p + sem_p
    return ((syn_p / den).astype(np.float32), syn_i,
            (len_p / den).astype(np.float32),
            (sem_p / den).astype(np.float32), sem_i, is_short)


def _windows(idx_sorted):
    """idx_sorted: [B, S] expert index per token, sorted ascending per row.
    Returns (re, caps): compile-time window starts and tile capacities
    covering bucket e on every core."""
    re, caps = [], []
    for e in range(3):
        starts = (idx_sorted < e).sum(axis=1)      # bucket start per core
        ends = (idx_sorted <= e).sum(axis=1)       # bucket end per core
        r = int(starts.min())
        hi = int(ends.max())
        re.append(r)
        caps.append(max(0, -(-(hi - r) // P)))     # ceil
    return tuple(re), tuple(caps)


# ---------------------------------------------------------------- device IR
def _build_program(geom):
    """geom = (re_syn, caps_syn, re_sem, caps_sem, synb_nz, lenb_nz, semb_nz)"""
    import concourse.bass as bass
    import concourse.tile as tile
    from concourse import bacc, masks, mybir

    re_syn, caps_syn, re_sem, caps_sem, synb_nz, lenb_nz, semb_nz = geom
    C_syn = sum(caps_syn)
    C_sem = sum(caps_sem)
    NT = C_syn + ST + C_sem                        # output tiles total
    f32 = mybir.dt.float32
    bf16 = mybir.dt.bfloat16
    AF = mybir.ActivationFunctionType
    ALU = mybir.AluOpType
    AX = mybir.AxisListType
    ts = bass.ts

    nc = bacc.Bacc("TRN2", target_bir_lowering=False, debug=False)

    # ---- DRAM I/O ----
    hsT_d = nc.dram_tensor("hsT", [H, S], bf16, kind="ExternalInput").ap()
    adjT_d = nc.dram_tensor("adjT", [S, S], bf16, kind="ExternalInput").ap()
    hs_d = nc.dram_tensor("hs", [S, H], f32, kind="ExternalInput").ap()
    semT_d = nc.dram_tensor("semT", [H, S], bf16, kind="ExternalInput").ap()
    wg1_d = nc.dram_tensor("wg1", [H, H], bf16, kind="ExternalInput").ap()
    wg2_d = nc.dram_tensor("wg2", [H, H], bf16, kind="ExternalInput").ap()
    wsyn_d = nc.dram_tensor("wsyn", [3, H, H], bf16, kind="ExternalInput").ap()
    wlen_d = nc.dram_tensor("wlen", [H, H], bf16, kind="ExternalInput").ap()
    wsem_d = nc.dram_tensor("wsem", [3, H, H], bf16, kind="ExternalInput").ap()
    wcls_d = nc.dram_tensor("wcls", [H, 2], bf16, kind="ExternalInput").ap()
    csyn_d = nc.dram_tensor("csyn", [max(C_syn, 1) * P], f32, kind="ExternalInput").ap()
    clen_d = nc.dram_tensor("clen", [S], f32, kind="ExternalInput").ap()
    csem_d = nc.dram_tensor("csem", [max(C_sem, 1) * P], f32, kind="ExternalInput").ap()
    bsyn_d = nc.dram_tensor("bsyn", [3, H], f32, kind="ExternalInput").ap() if synb_nz else None
    blen_d = nc.dram_tensor("blen", [1, H], f32, kind="ExternalInput").ap() if lenb_nz else None
    bsem_d = nc.dram_tensor("bsem", [3, H], f32, kind="ExternalInput").ap() if semb_nz else None
    out_d = nc.dram_tensor("out", [NT * P, 2], f32, kind="ExternalOutput").ap()

    hsT_r = hsT_d.rearrange("(k p) s -> p k s", p=P)
    adjT_r = adjT_d.rearrange("(t p) s -> p t s", p=P)
    hs_r = hs_d.rearrange("(a p) h -> p a h", p=P)
    semT_r = semT_d.rearrange("(k p) s -> p k s", p=P)
    wcls_r = wcls_d.rearrange("(k p) c -> p k c", p=P)
    csyn_r = csyn_d.rearrange("(a p) -> p a", p=P)
    clen_r = clen_d.rearrange("(a p) -> p a", p=P)
    csem_r = csem_d.rearrange("(a p) -> p a", p=P)
    out_r = out_d.rearrange("(a p) c -> p a c", p=P)

    with tile.TileContext(nc) as tc, ExitStack() as ctx:
        # ---- pools ----
        const = ctx.enter_context(tc.tile_pool(name="const", bufs=1))
        hsempool = ctx.enter_context(tc.tile_pool(name="hsempool", bufs=1))
        adjpool = ctx.enter_context(tc.tile_pool(name="adjpool", bufs=1))
        suppool = ctx.enter_context(tc.tile_pool(name="suppool", bufs=1))
        bigpool = ctx.enter_context(tc.tile_pool(name="bigpool", bufs=1))
        wpool = ctx.enter_context(tc.tile_pool(name="wpool", bufs=24))
        small = ctx.enter_context(tc.tile_pool(name="small", bufs=2))
        transb = ctx.enter_context(tc.tile_pool(name="transb", bufs=2))
        transf = ctx.enter_context(tc.tile_pool(name="transf", bufs=1))
        rowf32 = ctx.enter_context(tc.tile_pool(name="rowf32", bufs=1))
        chunkp = ctx.enter_context(tc.tile_pool(name="chunkp", bufs=3))
        acc = ctx.enter_context(tc.tile_pool(name="acc", bufs=4, space="PSUM"))
        tp = ctx.enter_context(tc.tile_pool(name="tp", bufs=2, space="PSUM"))
        clsps = ctx.enter_context(tc.tile_pool(name="clsps", bufs=2, space="PSUM"))

        # ---- constants / small inputs ----
        id_f32 = const.tile([P, P], f32, tag="idf")
        masks.make_identity(nc, id_f32[:])
        id_bf = const.tile([P, P], bf16, tag="idb")
        masks.make_identity(nc, id_bf[:])
        eps_t = const.tile([P, 1], f32, tag="eps")
        nc.vector.memset(eps_t[:], EPS)
        wcls_sb = const.tile([P, KT, 2], bf16, tag="wcls")
        nc.gpsimd.dma_start(wcls_sb[:], wcls_r)
        csyn_t = const.tile([P, max(C_syn, 1)], f32, tag="csyn")
        nc.gpsimd.dma_start(csyn_t[:], csyn_r)
        clen_t = const.tile([P, ST], f32, tag="clen")
        nc.gpsimd.dma_start(clen_t[:], clen_r)
        csem_t = const.tile([P, max(C_sem, 1)], f32, tag="csem")
        nc.gpsimd.dma_start(csem_t[:], csem_r)
        ones_row = None
        if synb_nz or lenb_nz or semb_nz:
            ones_row = const.tile([1, NCH], f32, tag="ones")
            nc.vector.memset(ones_row[:], 1.0)

        def bias_row(dram_ap, tag):
            t = const.tile([1, H], f32, tag=tag)
            nc.gpsimd.dma_start(t[:], dram_ap)
            return t

        bsyn_sb = ([bias_row(bsyn_d[e : e + 1, :], f"bsyn{e}") for e in range(3)]
                   if bsyn_d is not None else [None] * 3)
        blen_sb = bias_row(blen_d, "blen") if blen_d is not None else None
        bsem_sb = ([bias_row(bsem_d[e : e + 1, :], f"bsem{e}") for e in range(3)]
                   if bsem_d is not None else [None] * 3)

        # ---- big persistent SBUF tensors ----
        # hsT and (later) semT share one pool slot: hsT is dead after GCN1,
        # semT is only needed for the final sem expert group.
        hsT = hsempool.tile([P, KT, S + SPAD], bf16, tag="hsem")
        nc.sync.dma_start(hsT[:, :, :S], hsT_r)
        bigT = bigpool.tile([P, KT, S + SPAD], bf16, tag="bigT")
        for k in range(KT):
            nc.vector.memset(bigT[:, k, S:], 0.0)
        sup = suppool.tile([P, ST, H], bf16, tag="sup")
        out_sb = small.tile([P, NT, 2], f32, tag="outsb")

        def load_wtiles(wdram):
            tiles = []
            for k in range(KT):
                wt = wpool.tile([P, H], bf16, tag="w")
                nc.sync.dma_start(wt[:], wdram[ts(k, P), :])
                tiles.append(wt)
            return tiles

        def transpose_into(dstT, src_of_k, m, ident):
            for k in range(KT):
                pt = tp.tile([P, P], ident.dtype, tag="tp")
                nc.tensor.transpose(pt[:], src_of_k(k), ident[:])
                nc.any.tensor_copy(dstT[:, k, ts(m, P)], pt[:])

        # ---- expert group helper (transposed form, fused cls) ----
        # One "window" = (rhsT, rhs_col0, ntiles, weight dram, bias, coef tile
        # column offset, out_sb tile offset). Emits, per (d, chunk):
        # expert matmuls -> [pipelined cls of the previous chunk] -> gelu.
        # cls partials accumulate over d into an SBUF accumulator; the
        # per-token coefficient is applied after cls (it factors out of the
        # d-contraction), so padding/masked slots simply get multiplied by 0.
        def expert_window(wdram, rhsT, rhs_col0, ntiles, bias_sb, coef_t,
                          coef_off, out_tile0):
            wt = load_wtiles(wdram)
            width = ntiles * P
            nch = [(c0, min(NCH, width - c0)) for c0 in range(0, width, NCH)]
            clsacc = small.tile([P, ntiles, 2], f32, tag="clsacc")
            nc.vector.memset(clsacc[:], 0.0)
            pending = None

            def emit_pending(p):
                ch, pc0, pw, pd = p
                nj = pw // P
                psd = clsps.tile([P, nj, 2], f32, tag="cls")
                for jj in range(nj):
                    nc.tensor.matmul(psd[:, jj, :], ch[:, ts(jj, P)],
                                     wcls_sb[:, pd, :],
                                     start=True, stop=True)
                j0 = pc0 // P
                nc.vector.tensor_add(clsacc[:, j0 : j0 + nj, :],
                                     clsacc[:, j0 : j0 + nj, :], psd[:])

            for d in range(KT):
                for c0, w in nch:
                    ps = acc.tile([P, NCH], f32, tag="acc")
                    for k in range(KT):
                        last = (k == KT - 1) and (bias_sb is None)
                        nc.tensor.matmul(ps[:, :w], wt[k][:, ts(d, P)],
                                         rhsT[:, k, rhs_col0 + c0 : rhs_col0 + c0 + w],
                                         start=(k == 0), stop=last)
                    if bias_sb is not None:
                        nc.tensor.matmul(ps[:, :w], bias_sb[:, ts(d, P)],
                                         ones_row[:, :w], start=False, stop=True)
                    if pending is not None:
                        emit_pending(pending)
                    ch = chunkp.tile([P, NCH], bf16, tag="ch")
                    nc.scalar.activation(ch[:, :w], ps[:, :w], AF.Gelu)
                    pending = (ch, c0, w, d)
            emit_pending(pending)
            for j in range(ntiles):
                nc.vector.tensor_scalar_mul(out_sb[:, out_tile0 + j, :],
                                            clsacc[:, j, :],
                                            coef_t[:, coef_off + j : coef_off + j + 1])

        # ---- len expert first (frees hsT for semT reuse) ----
        expert_window(wlen_d, hsT, 0, ST, blen_sb, clen_t, 0, C_syn)

        # ---- GCN layer 1: sup1 = hs @ Wg1 (k-outer: start on first tiles) ----
        w_g1 = load_wtiles(wg1_d)
        for n in range(NN):
            for mb in (range(0, 3), range(3, 6), range(6, 8)):
                pss = {}
                for m in mb:
                    psk = acc.tile([P, NCH], f32, tag="acc")
                    pss[m] = psk
                for k in range(KT):
                    for m in mb:
                        nc.tensor.matmul(pss[m][:], hsT[:, k, ts(m, P)],
                                         w_g1[k][:, ts(n, NCH)],
                                         start=(k == 0), stop=(k == KT - 1))
                for m in mb:
                    nc.any.tensor_copy(sup[:, m, ts(n, NCH)], pss[m][:])

        # ---- agg1: x1 = relu(A_hat @ sup1), transposed into bigT ----
        adjT = adjpool.tile([P, TT, S], bf16, tag="adjT")
        nc.sync.dma_start(adjT[:], adjT_r)
        # semT replaces hsT in the shared slot (loads during GCN phases)
        semT = hsempool.tile([P, KT, S + SPAD], bf16, tag="hsem")
        for k in range(KT):
            nc.vector.memset(semT[:, k, S:], 0.0)
        nc.sync.dma_start(semT[:, :, :S], semT_r)
        w_g2 = load_wtiles(wg2_d)
        for m in range(ST):
            x1row = transb.tile([P, H], bf16, tag="xrow")
            for n in range(NN):
                ps = acc.tile([P, NCH], f32, tag="acc")
                for t in range(TT):
                    nc.tensor.matmul(ps[:], adjT[:, t, ts(m, P)],
                                     sup[:, t, ts(n, NCH)],
                                     start=(t == 0), stop=(t == TT - 1))
                nc.scalar.activation(x1row[:, ts(n, NCH)], ps[:], AF.Relu)
            transpose_into(bigT, lambda k: x1row[:, ts(k, P)], m, id_bf)

        # ---- GCN layer 2: sup2 = x1 @ Wg2 ----
        for m in range(ST):
            for n in range(NN):
                ps = acc.tile([P, NCH], f32, tag="acc")
                for k in range(KT):
                    nc.tensor.matmul(ps[:], bigT[:, k, ts(m, P)],
                                     w_g2[k][:, ts(n, NCH)],
                                     start=(k == 0), stop=(k == KT - 1))
                nc.any.tensor_copy(sup[:, m, ts(n, NCH)], ps[:])

        # ---- agg2 + residual + LayerNorm -> sharedT (into bigT) ----
        for m in range(ST):
            hsm = rowf32.tile([P, H], f32, tag="hsm")
            nc.sync.dma_start(hsm[:], hs_r[:, m, :])
            x2row = transf.tile([P, H], f32, tag="x2row")
            for n in range(NN):
                ps = acc.tile([P, NCH], f32, tag="acc")
                for t in range(TT):
                    nc.tensor.matmul(ps[:], adjT[:, t, ts(m, P)],
                                     sup[:, t, ts(n, NCH)],
                                     start=(t == 0), stop=(t == TT - 1))
                nc.scalar.activation(x2row[:, ts(n, NCH)], ps[:], AF.Relu)
            nc.vector.tensor_add(x2row[:], x2row[:], hsm[:])
            stats = small.tile([P, NN, 6], f32, tag="stats")
            for c in range(NN):
                nc.vector.bn_stats(stats[:, c, :], x2row[:, ts(c, NCH)])
            mv = small.tile([P, 2], f32, tag="mv")
            nc.vector.bn_aggr(mv[:], stats[:])
            rstd = small.tile([P, 1], f32, tag="rstd")
            nc.scalar.activation(rstd[:], mv[:, 1:2], AF.Sqrt, bias=eps_t[:])
            nc.vector.reciprocal(rstd[:], rstd[:])
            nc.vector.tensor_scalar(out=x2row[:], in0=x2row[:],
                                    scalar1=mv[:, 0:1], scalar2=rstd[:],
                                    op0=ALU.subtract, op1=ALU.mult)
            transpose_into(bigT, lambda k: x2row[:, ts(k, P)], m, id_f32)

        # ---- syn experts on sharedT (bigT) ----
        fo = 0
        for e in range(3):
            if caps_syn[e]:
                expert_window(wsyn_d[e], bigT, re_syn[e], caps_syn[e],
                              bsyn_sb[e], csyn_t, fo, fo)
                fo += caps_syn[e]

        # ---- sem experts on semT ----
        fo = 0
        for e in range(3):
            if caps_sem[e]:
                expert_window(wsem_d[e], semT, re_sem[e], caps_sem[e],
                              bsem_sb[e], csem_t, fo, C_syn + ST + fo)
                fo += caps_sem[e]

        nc.gpsimd.dma_start(out_r, out_sb[:])

    nc.compile()
    return nc


def _get_program(geom):
    if geom not in _prog_cache:
        _prog_cache[geom] = _build_program(geom)
    return _prog_cache[geom]


# ---------------------------------------------------------------- host glue
def _prepare(inputs):
    """Compute routing, permutations, windows; build per-core in_maps and
    decode metadata. Returns (geom, in_maps, meta)."""
    hs = np.asarray(inputs["hidden_states"], dtype=np.float32)
    adj = np.asarray(inputs["adj_matrix"], dtype=np.float32)
    seq_lengths = np.asarray(inputs["seq_lengths"])
    router_w = np.asarray(inputs["router_w"], dtype=np.float32)
    router_b = np.asarray(inputs["router_b"], dtype=np.float32)
    gcn1_w = np.asarray(inputs["gcn1_w"], dtype=np.float32)
    gcn2_w = np.asarray(inputs["gcn2_w"], dtype=np.float32)
    ln_g = np.asarray(inputs["ln_g"], dtype=np.float32)
    ln_b = np.asarray(inputs["ln_b"], dtype=np.float32)
    syn_w = np.asarray(inputs["syn_w"], dtype=np.float32)
    syn_b = np.asarray(inputs["syn_b"], dtype=np.float32)
    len_short_w = np.asarray(inputs["len_short_w"], dtype=np.float32)
    len_short_b = np.asarray(inputs["len_short_b"], dtype=np.float32)
    len_long_w = np.asarray(inputs["len_long_w"], dtype=np.float32)
    len_long_b = np.asarray(inputs["len_long_b"], dtype=np.float32)
    sem_w = np.asarray(inputs["sem_w"], dtype=np.float32)
    sem_b = np.asarray(inputs["sem_b"], dtype=np.float32)
    cls_w = np.asarray(inputs["cls_w"], dtype=np.float32)
    cls_b = np.asarray(inputs["cls_b"], dtype=np.float32)

    c_syn, syn_i, c_len, c_sem, sem_i, is_short = _route_host(
        hs, router_w, router_b, seq_lengths)

    # fold LN affine into syn weights: LN_plain(x) @ (g*W) + (b@W + bias)
    syn_w_f = (ln_g[None, :, None] * syn_w).astype(np.float32)
    syn_b_f = (syn_b + np.einsum("h,ehd->ed", ln_b, syn_w)).astype(np.float32)

    perm = np.argsort(syn_i, axis=1, kind="stable")          # [B,S]
    syn_i_p = np.take_along_axis(syn_i, perm, axis=1)
    sem_i_p = np.take_along_axis(sem_i, perm, axis=1)
    sem_perm = np.argsort(sem_i_p, axis=1, kind="stable")    # syn-order -> sem-order
    sem_i_s = np.take_along_axis(sem_i_p, sem_perm, axis=1)

    re_syn, caps_syn = _windows(syn_i_p)
    re_sem, caps_sem = _windows(sem_i_s)
    C_syn, C_sem = sum(caps_syn), sum(caps_sem)

    synb_nz = bool(np.any(syn_b_f != 0))
    lenb_nz = bool(np.any(len_short_b != 0) or np.any(len_long_b != 0))
    semb_nz = bool(np.any(sem_b != 0))
    geom = (re_syn, caps_syn, re_sem, caps_sem, synb_nz, lenb_nz, semb_nz)

    wg1 = np.ascontiguousarray(gcn1_w.astype(_BF16))
    wg2 = np.ascontiguousarray(gcn2_w.astype(_BF16))
    wsyn = np.ascontiguousarray(syn_w_f.astype(_BF16))
    wlen_s = np.ascontiguousarray(len_short_w.astype(_BF16))
    wlen_l = np.ascontiguousarray(len_long_w.astype(_BF16))
    wsem = np.ascontiguousarray(sem_w.astype(_BF16))
    wcls = np.ascontiguousarray(cls_w.astype(_BF16))

    def win_coef(cvec, idx_sorted_row, re, caps, grp):
        """Per-window masked coefficients, zero-padded to caps*P."""
        out = np.zeros(max(sum(caps), 1) * P, np.float32)
        off = 0
        for e in range(3):
            w = caps[e] * P
            lo = re[e]
            hi = min(S, lo + w)
            seg = np.where(idx_sorted_row[lo:hi] == e, cvec[lo:hi], 0.0)
            out[off : off + (hi - lo)] = seg
            off += w
        return out

    in_maps = []
    meta = []
    for b in range(B):
        p = perm[b]
        sp = sem_perm[b]
        hs_p = hs[b][p]
        adj_p = adj[b][p][:, p]
        deg = np.maximum(adj_p.sum(axis=1, dtype=np.float32), 1e-9)
        adj_n = adj_p / deg[:, None]
        hs_sem = hs_p[sp]
        c_syn_p = c_syn[b][p]
        c_len_p = c_len[b][p]
        c_sem_s = c_sem[b][p][sp]

        m = {
            "hsT": np.ascontiguousarray(hs_p.T.astype(_BF16)),
            "adjT": np.ascontiguousarray(adj_n.T.astype(_BF16)),
            "hs": np.ascontiguousarray(hs_p),
            "semT": np.ascontiguousarray(hs_sem.T.astype(_BF16)),
            "wg1": wg1, "wg2": wg2, "wsyn": wsyn,
            "wlen": wlen_s if is_short[b] else wlen_l,
            "wsem": wsem, "wcls": wcls,
            "csyn": win_coef(c_syn_p, syn_i_p[b], re_syn, caps_syn, "syn"),
            "clen": np.ascontiguousarray(c_len_p),
            "csem": win_coef(c_sem_s, sem_i_s[b], re_sem, caps_sem, "sem"),
        }
        if synb_nz:
            m["bsyn"] = syn_b_f
        if lenb_nz:
            m["blen"] = (len_short_b if is_short[b]
                         else len_long_b).reshape(1, H).astype(np.float32)
        if semb_nz:
            m["bsem"] = sem_b.astype(np.float32)
        in_maps.append(m)
        meta.append((p, sp))

    return geom, in_maps, meta, cls_b


def _decode(out_rows, geom, meta_b, cls_b):
    """out_rows: [NT*P, 2] device output for one core -> [S,2] original order."""
    re_syn, caps_syn, re_sem, caps_sem = geom[0], geom[1], geom[2], geom[3]
    C_syn, C_sem = sum(caps_syn), sum(caps_sem)
    p, sp = meta_b
    acc_syn = np.zeros((S, 2), np.float32)   # syn-order accumulation
    off = 0
    for e in range(3):
        w = caps_syn[e] * P
        lo = re_syn[e]
        hi = min(S, lo + w)
        acc_syn[lo:hi] += out_rows[off : off + (hi - lo)]
        off += w
    acc_syn += out_rows[C_syn * P : C_syn * P + S]          # len group
    acc_sem = np.zeros((S, 2), np.float32)   # sem-order
    off = (C_syn + ST) * P
    for e in range(3):
        w = caps_sem[e] * P
        lo = re_sem[e]
        hi = min(S, lo + w)
        acc_sem[lo:hi] += out_rows[off : off + (hi - lo)]
        off += w
    acc_syn[sp] += acc_sem
    res = np.empty((S, 2), np.float32)
    res[p] = acc_syn
    return res + cls_b


def kernel(**inputs):
    from concourse import bass_utils

    geom, in_maps, meta, cls_b = _prepare(inputs)
    nc = _get_program(geom)

    try:
        res = bass_utils.run_bass_kernel_spmd(nc, in_maps, core_ids=list(range(B)))
    except Exception:
        # transient device wedge (NRT_EXEC_UNIT_UNRECOVERABLE) clears on retry
        res = bass_utils.run_bass_kernel_spmd(nc, in_maps, core_ids=list(range(B)))
    globals()["_last_results"] = res
    out = np.stack([_decode(np.asarray(res.results[b]["out"], np.float32),
                            geom, meta[b], cls_b)
                    for b in range(B)]).astype(np.float32)
    return out


# revision 21
# speedup vs baseline: 1.7525x; 1.1071x over previous
"""Trainium2 Bass kernel for nn_MoEDetector (moe_routing).

Strategy: data-parallel over batch B=8 -> one batch per NeuronCore, plus
top-1 expert bucketing so the syn/sem groups run ~3/8 of the dense work.

Host side (cheap, exact):
  - router logits/probs/argmax + group coefficients in fp32 numpy
    (top-2 logit gaps are ~1e-4 while fp32 sum-order noise is ~1e-6, so
    the argmax always matches the jax reference)
  - tokens sorted by syn expert (perm applied to hs, adj rows+cols);
    second sort by sem expert gives hs_sem
  - per-expert compile-time column WINDOWS [re_e, re_e+cap_e*128) that
    cover the bucket on every core (offsets differ per core; the window
    union is compile-time, per-core masking via zeroed coefficients)
  - adjacency degree-normalized + transposed, hs transposed, both bf16
  - final per-group cls outputs are unpermuted and summed on host

Device program (shared by all 8 cores; per-core tensor CONTENT differs):
  - GCN1 -> agg1(relu) -> GCN2 -> agg2(relu) -> +hs residual -> LayerNorm
    (affine folded into syn weights), matmuls bf16, accumulation fp32
  - experts run TRANSPOSED: out_T[d,tok] = W^T @ x_T, so the gelu output
    lands pre-transposed for the cls projection and the per-token
    coefficient factors out of the d-contraction -> applied after cls as
    a per-partition scalar on the [slots,2] result
  - groups: syn (3 windows on sharedT), len (all tokens on hsT),
    sem (3 windows on hs_semT); each -> fusedT bf16 -> cls -> out rows
"""

import numpy as np
import ml_dtypes
from contextlib import ExitStack

B, S, H = 8, 1024, 1536
THRESHOLD = 128
NEG = -1e9
P = 128
ST = S // P          # 8 token tiles
KT = H // P          # 12 h tiles
TT = S // P          # 8 t tiles
NCH = 512            # matmul moving free-dim chunk
NN = H // NCH        # 3 chunks of H
EPS = 1e-5
SPAD = 384           # pad tail so expert windows may overrun S

_BF16 = ml_dtypes.bfloat16

_prog_cache = {}


# ---------------------------------------------------------------- host math
def _route_host(hs, rw, rb, seq_lengths):
    """fp32 numpy replication of the reference router."""
    logits = (hs.reshape(-1, H).astype(np.float32) @ rw).reshape(B, S, 8) + rb
    is_short = (np.asarray(seq_lengths) <= THRESHOLD)
    lg = logits.copy()
    lg[..., 3] = np.where(is_short[:, None], logits[..., 3], NEG)
    lg[..., 4] = np.where(is_short[:, None], NEG, logits[..., 4])
    m = lg.max(-1, keepdims=True)
    e = np.exp((lg - m).astype(np.float32))
    probs = (e / e.sum(-1, keepdims=True)).astype(np.float32)
    syn_p = probs[..., 0:3].max(-1)
    syn_i = probs[..., 0:3].argmax(-1)
    len_p = probs[..., 3:5].max(-1)
    sem_p = probs[..., 5:8].max(-1)
    sem_i = probs[..., 5:8].argmax(-1)
    den = syn_p + len_p + sem_p
    return ((syn_p / den).astype(np.float32), syn_i,
            (len_p / den).astype(np.float32),
            (sem_p / den).astype(np.float32), sem_i, is_short)


def _windows(idx_sorted):
    """idx_sorted: [B, S] expert index per token, sorted ascending per row.
    Returns (re, caps): compile-time window starts and tile capacities
    covering bucket e on every core."""
    re, caps = [], []
    for e in range(3):
        starts = (idx_sorted < e).sum(axis=1)      # bucket start per core
        ends = (idx_sorted <= e).sum(axis=1)       # bucket end per core
        r = int(starts.min())
        hi = int(ends.max())
        re.append(r)
        caps.append(max(0, -(-(hi - r) // P)))     # ceil
    return tuple(re), tuple(caps)


# ---------------------------------------------------------------- device IR
def _build_program(geom):
    """geom = (re_syn, caps_syn, re_sem, caps_sem, synb_nz, lenb_nz, semb_nz)"""
    import concourse.bass as bass
    import concourse.tile as tile
    from concourse import bacc, masks, mybir

    re_syn, caps_syn, re_sem, caps_sem, synb_nz, lenb_nz, semb_nz = geom
    C_syn = sum(caps_syn)
    C_sem = sum(caps_sem)
    NT = C_syn + ST + C_sem                        # output tiles total
    spad_syn = max([0] + [re_syn[e] + caps_syn[e] * P - S for e in range(3)])
    spad_sem = max([0] + [re_sem[e] + caps_sem[e] * P - S for e in range(3)])
    f32 = mybir.dt.float32
    bf16 = mybir.dt.bfloat16
    AF = mybir.ActivationFunctionType
    ALU = mybir.AluOpType
    AX = mybir.AxisListType
    ts = bass.ts

    nc = bacc.Bacc("TRN2", target_bir_lowering=False, debug=False)

    # ---- DRAM I/O ----
    hsT_d = nc.dram_tensor("hsT", [H, S], bf16, kind="ExternalInput").ap()
    adjT_d = nc.dram_tensor("adjT", [S, S], bf16, kind="ExternalInput").ap()
    hs_d = nc.dram_tensor("hs", [S, H], f32, kind="ExternalInput").ap()
    semT_d = nc.dram_tensor("semT", [H, S], bf16, kind="ExternalInput").ap()
    wg1_d = nc.dram_tensor("wg1", [H, H], bf16, kind="ExternalInput").ap()
    wg2_d = nc.dram_tensor("wg2", [H, H], bf16, kind="ExternalInput").ap()
    wsyn_d = nc.dram_tensor("wsyn", [3, H, H], bf16, kind="ExternalInput").ap()
    wlen_d = nc.dram_tensor("wlen", [H, H], bf16, kind="ExternalInput").ap()
    wsem_d = nc.dram_tensor("wsem", [3, H, H], bf16, kind="ExternalInput").ap()
    wcls_d = nc.dram_tensor("wcls", [H, 2], bf16, kind="ExternalInput").ap()
    csyn_d = nc.dram_tensor("csyn", [max(C_syn, 1) * P], f32, kind="ExternalInput").ap()
    clen_d = nc.dram_tensor("clen", [S], f32, kind="ExternalInput").ap()
    csem_d = nc.dram_tensor("csem", [max(C_sem, 1) * P], f32, kind="ExternalInput").ap()
    bsyn_d = nc.dram_tensor("bsyn", [3, H], f32, kind="ExternalInput").ap() if synb_nz else None
    blen_d = nc.dram_tensor("blen", [1, H], f32, kind="ExternalInput").ap() if lenb_nz else None
    bsem_d = nc.dram_tensor("bsem", [3, H], f32, kind="ExternalInput").ap() if semb_nz else None
    out_d = nc.dram_tensor("out", [NT * P, 2], f32, kind="ExternalOutput").ap()

    hsT_r = hsT_d.rearrange("(k p) s -> p k s", p=P)
    adjT_r = adjT_d.rearrange("(t p) s -> p t s", p=P)
    hs_r = hs_d.rearrange("(a p) h -> p a h", p=P)
    semT_r = semT_d.rearrange("(k p) s -> p k s", p=P)
    wcls_r = wcls_d.rearrange("(k p) c -> p k c", p=P)
    csyn_r = csyn_d.rearrange("(a p) -> p a", p=P)
    clen_r = clen_d.rearrange("(a p) -> p a", p=P)
    csem_r = csem_d.rearrange("(a p) -> p a", p=P)
    out_r = out_d.rearrange("(a p) c -> p a c", p=P)

    with tile.TileContext(nc) as tc, ExitStack() as ctx:
        # ---- pools ----
        const = ctx.enter_context(tc.tile_pool(name="const", bufs=1))
        hsempool = ctx.enter_context(tc.tile_pool(name="hsempool", bufs=1))
        adjpool = ctx.enter_context(tc.tile_pool(name="adjpool", bufs=1))
        suppool = ctx.enter_context(tc.tile_pool(name="suppool", bufs=1))
        bigpool = ctx.enter_context(tc.tile_pool(name="bigpool", bufs=1))
        wpool = ctx.enter_context(tc.tile_pool(name="wpool", bufs=24))
        small = ctx.enter_context(tc.tile_pool(name="small", bufs=2))
        transb = ctx.enter_context(tc.tile_pool(name="transb", bufs=2))
        transf = ctx.enter_context(tc.tile_pool(name="transf", bufs=2))
        rowf32 = ctx.enter_context(tc.tile_pool(name="rowf32", bufs=1))
        chunkp = ctx.enter_context(tc.tile_pool(name="chunkp", bufs=3))
        acc = ctx.enter_context(tc.tile_pool(name="acc", bufs=5, space="PSUM"))
        tp = ctx.enter_context(tc.tile_pool(name="tp", bufs=2, space="PSUM"))
        clsps = ctx.enter_context(tc.tile_pool(name="clsps", bufs=1, space="PSUM"))

        # ---- constants / small inputs ----
        id_f32 = const.tile([P, P], f32, tag="idf")
        masks.make_identity(nc, id_f32[:])
        id_bf = const.tile([P, P], bf16, tag="idb")
        masks.make_identity(nc, id_bf[:])
        eps_t = const.tile([P, 1], f32, tag="eps")
        nc.vector.memset(eps_t[:], EPS)
        wcls_sb = const.tile([P, KT, 2], bf16, tag="wcls")
        nc.gpsimd.dma_start(wcls_sb[:], wcls_r)
        csyn_t = const.tile([P, max(C_syn, 1)], f32, tag="csyn")
        nc.gpsimd.dma_start(csyn_t[:], csyn_r)
        clen_t = const.tile([P, ST], f32, tag="clen")
        nc.gpsimd.dma_start(clen_t[:], clen_r)
        csem_t = const.tile([P, max(C_sem, 1)], f32, tag="csem")
        nc.gpsimd.dma_start(csem_t[:], csem_r)
        ones_row = None
        if synb_nz or lenb_nz or semb_nz:
            ones_row = const.tile([1, NCH], f32, tag="ones")
            nc.vector.memset(ones_row[:], 1.0)

        def bias_row(dram_ap, tag):
            t = const.tile([1, H], f32, tag=tag)
            nc.gpsimd.dma_start(t[:], dram_ap)
            return t

        bsyn_sb = ([bias_row(bsyn_d[e : e + 1, :], f"bsyn{e}") for e in range(3)]
                   if bsyn_d is not None else [None] * 3)
        blen_sb = bias_row(blen_d, "blen") if blen_d is not None else None
        bsem_sb = ([bias_row(bsem_d[e : e + 1, :], f"bsem{e}") for e in range(3)]
                   if bsem_d is not None else [None] * 3)

        # ---- big persistent SBUF tensors ----
        # hsT and (later) semT share one pool slot: hsT is dead after GCN1,
        # semT is only needed for the final sem expert group.
        hsT = hsempool.tile([P, KT, S + spad_sem], bf16, tag="hsem")
        bigT = bigpool.tile([P, KT, S + spad_syn], bf16, tag="bigT")
        for k in range(KT):
            if spad_syn:
                nc.vector.memset(bigT[:, k, S:], 0.0)
        sup = suppool.tile([P, ST, H], bf16, tag="sup")
        out_sb = small.tile([P, NT, 2], f32, tag="outsb")

        def load_wtiles(wdram):
            tiles = []
            for k in range(KT):
                wt = wpool.tile([P, H], bf16, tag="w")
                nc.sync.dma_start(wt[:], wdram[ts(k, P), :])
                tiles.append(wt)
            return tiles

        def transpose_into(dstT, src_of_k, m, ident):
            for k in range(KT):
                pt = tp.tile([P, P], ident.dtype, tag="tp")
                nc.tensor.transpose(pt[:], src_of_k(k), ident[:])
                nc.any.tensor_copy(dstT[:, k, ts(m, P)], pt[:])

        # ---- expert group helper (transposed form, fused cls) ----
        # One "window" = (rhsT, rhs_col0, ntiles, weight dram, bias, coef tile
        # column offset, out_sb tile offset). Emits, per (d, chunk):
        # expert matmuls -> [pipelined cls of the previous chunk] -> gelu.
        # cls partials accumulate over d into an SBUF accumulator; the
        # per-token coefficient is applied after cls (it factors out of the
        # d-contraction), so padding/masked slots simply get multiplied by 0.
        def expert_window(wdram, rhsT, rhs_col0, ntiles, bias_sb, coef_t,
                          coef_off, out_tile0, dma_with=None, stream_first=0):
            wt = []
            for k in range(KT):
                if dma_with is not None:
                    dma_with(k)
                wk = wpool.tile([P, H], bf16, tag="w")
                nc.sync.dma_start(wk[:], wdram[ts(k, P), :])
                wt.append(wk)
            width = ntiles * P
            nch = [(c0, min(NCH, width - c0)) for c0 in range(0, width, NCH)]
            groups = [(d, c0, w) for d in range(KT) for (c0, w) in nch]
            clsacc = small.tile([P, ntiles, 2], f32, tag="clsacc")
            nc.vector.memset(clsacc[:], 0.0)
            pending = None

            def emit_pending(p):
                ch, pc0, pw, pd = p
                nj = pw // P
                psd = clsps.tile([P, nj, 2], f32, tag="cls")
                for jj in range(nj):
                    nc.tensor.matmul(psd[:, jj, :], ch[:, ts(jj, P)],
                                     wcls_sb[:, pd, :],
                                     start=True, stop=True)
                j0 = pc0 // P
                nc.vector.tensor_add(clsacc[:, j0 : j0 + nj, :],
                                     clsacc[:, j0 : j0 + nj, :], psd[:])

            def finish_group(ps, d, c0, w):
                nonlocal pending
                if bias_sb is not None:
                    nc.tensor.matmul(ps[:, :w], bias_sb[:, ts(d, P)],
                                     ones_row[:, :w], start=False, stop=True)
                if pending is not None:
                    emit_pending(pending)
                ch = chunkp.tile([P, NCH], bf16, tag="ch")
                nc.scalar.activation(ch[:, :w], ps[:, :w], AF.Gelu)
                pending = (ch, c0, w, d)

            gi = 0
            if stream_first > 1:
                # k-outer over the first few groups so matmuls overlap the
                # initial weight/activation DMA stream tile-by-tile
                blk = groups[:stream_first]
                pss = []
                for _ in blk:
                    psk = acc.tile([P, NCH], f32, tag="acc")
                    pss.append(psk)
                for k in range(KT):
                    for ps, (d, c0, w) in zip(pss, blk):
                        last = (k == KT - 1) and (bias_sb is None)
                        nc.tensor.matmul(ps[:, :w], wt[k][:, ts(d, P)],
                                         rhsT[:, k, rhs_col0 + c0 : rhs_col0 + c0 + w],
                                         start=(k == 0), stop=last)
                for ps, (d, c0, w) in zip(pss, blk):
                    finish_group(ps, d, c0, w)
                gi = stream_first
            for d, c0, w in groups[gi:]:
                ps = acc.tile([P, NCH], f32, tag="acc")
                for k in range(KT):
                    last = (k == KT - 1) and (bias_sb is None)
                    nc.tensor.matmul(ps[:, :w], wt[k][:, ts(d, P)],
                                     rhsT[:, k, rhs_col0 + c0 : rhs_col0 + c0 + w],
                                     start=(k == 0), stop=last)
                finish_group(ps, d, c0, w)
            emit_pending(pending)
            pending = None
            for j in range(ntiles):
                nc.vector.tensor_scalar_mul(out_sb[:, out_tile0 + j, :],
                                            clsacc[:, j, :],
                                            coef_t[:, coef_off + j : coef_off + j + 1])

        # ---- len expert first (frees hsT for semT reuse) ----
        def dma_hsT_k(k):
            nc.sync.dma_start(hsT[:, k, :S], hsT_r[:, k, :])

        expert_window(wlen_d, hsT, 0, ST, blen_sb, clen_t, 0, C_syn,
                      dma_with=dma_hsT_k, stream_first=5)
        nc.gpsimd.dma_start(out_r[:, C_syn : C_syn + ST, :],
                            out_sb[:, C_syn : C_syn + ST, :])

        # ---- GCN layer 1: sup1 = hs @ Wg1 (k-outer: start on first tiles) ----
        w_g1 = load_wtiles(wg1_d)
        for n in range(NN):
            for mb in (range(0, 3), range(3, 6), range(6, 8)):
                pss = {}
                for m in mb:
                    psk = acc.tile([P, NCH], f32, tag="acc")
                    pss[m] = psk
                for k in range(KT):
                    for m in mb:
                        nc.tensor.matmul(pss[m][:], hsT[:, k, ts(m, P)],
                                         w_g1[k][:, ts(n, NCH)],
                                         start=(k == 0), stop=(k == KT - 1))
                for m in mb:
                    nc.any.tensor_copy(sup[:, m, ts(n, NCH)], pss[m][:])

        # ---- agg1 (transposed): x1T[d,tok] = relu(sup1^T @ A_hat^T) ----
        # lhsT = sup1 tile (contraction over source tokens on partitions),
        # rhs = adjT tile; the agg matmul itself produces x1T -> no transposes
        adjT = adjpool.tile([P, TT, S], bf16, tag="adjT")
        nc.sync.dma_start(adjT[:], adjT_r)
        # semT replaces hsT in the shared slot (loads during GCN phases)
        semT = hsempool.tile([P, KT, S + spad_sem], bf16, tag="hsem")
        for k in range(KT):
            if spad_sem:
                nc.vector.memset(semT[:, k, S:], 0.0)
        nc.sync.dma_start(semT[:, :, :S], semT_r)
        w_g2 = load_wtiles(wg2_d)
        for d in range(KT):
            for c in range(S // NCH):
                ps = acc.tile([P, NCH], f32, tag="acc")
                for t in range(TT):
                    nc.tensor.matmul(ps[:], sup[:, t, ts(d, P)],
                                     adjT[:, t, ts(c, NCH)],
                                     start=(t == 0), stop=(t == TT - 1))
                nc.scalar.activation(bigT[:, d, ts(c, NCH)], ps[:], AF.Relu)

        # ---- GCN layer 2: sup2 = x1 @ Wg2 ----
        for m in range(ST):
            for n in range(NN):
                ps = acc.tile([P, NCH], f32, tag="acc")
                for k in range(KT):
                    nc.tensor.matmul(ps[:], bigT[:, k, ts(m, P)],
                                     w_g2[k][:, ts(n, NCH)],
                                     start=(k == 0), stop=(k == KT - 1))
                nc.any.tensor_copy(sup[:, m, ts(n, NCH)], ps[:])

        # ---- agg2 + residual + LayerNorm -> sharedT (into bigT) ----
        # Transposes run one m behind so the LN chain (DVE/Act) overlaps the
        # next tile's agg matmuls instead of stalling PE.
        prev_xb = None
        for m in range(ST):
            hsm = rowf32.tile([P, H], f32, tag="hsm")
            nc.sync.dma_start(hsm[:], hs_r[:, m, :])
            x2row = transf.tile([P, H], f32, tag="x2row")
            for n in range(NN):
                ps = acc.tile([P, NCH], f32, tag="acc")
                for t in range(TT):
                    nc.tensor.matmul(ps[:], adjT[:, t, ts(m, P)],
                                     sup[:, t, ts(n, NCH)],
                                     start=(t == 0), stop=(t == TT - 1))
                nc.scalar.activation(x2row[:, ts(n, NCH)], ps[:], AF.Relu)
                if n == 0 and prev_xb is not None:
                    xb, pm = prev_xb
                    transpose_into(bigT, lambda k: xb[:, ts(k, P)], pm, id_bf)
            nc.vector.tensor_add(x2row[:], x2row[:], hsm[:])
            stats = small.tile([P, NN, 6], f32, tag="stats")
            for c in range(NN):
                nc.vector.bn_stats(stats[:, c, :], x2row[:, ts(c, NCH)])
            mv = small.tile([P, 2], f32, tag="mv")
            nc.vector.bn_aggr(mv[:], stats[:])
            rstd = small.tile([P, 1], f32, tag="rstd")
            nc.scalar.activation(rstd[:], mv[:, 1:2], AF.Sqrt, bias=eps_t[:])
            nc.vector.reciprocal(rstd[:], rstd[:])
            xb = transb.tile([P, H], bf16, tag="xrow")
            nc.vector.tensor_scalar(out=xb[:], in0=x2row[:],
                                    scalar1=mv[:, 0:1], scalar2=rstd[:],
                                    op0=ALU.subtract, op1=ALU.mult)
            prev_xb = (xb, m)
        xb, pm = prev_xb
        transpose_into(bigT, lambda k: xb[:, ts(k, P)], pm, id_bf)

        # ---- syn experts on sharedT (bigT) ----
        fo = 0
        for e in range(3):
            if caps_syn[e]:
                expert_window(wsyn_d[e], bigT, re_syn[e], caps_syn[e],
                              bsyn_sb[e], csyn_t, fo, fo)
                fo += caps_syn[e]
        if C_syn:
            nc.gpsimd.dma_start(out_r[:, 0:C_syn, :], out_sb[:, 0:C_syn, :])

        # ---- sem experts on semT ----
        fo = 0
        for e in range(3):
            if caps_sem[e]:
                expert_window(wsem_d[e], semT, re_sem[e], caps_sem[e],
                              bsem_sb[e], csem_t, fo, C_syn + ST + fo)
                fo += caps_sem[e]
            nc.gpsimd.dma_start(
                out_r[:, C_syn + ST + fo - caps_sem[e] : C_syn + ST + fo, :],
                out_sb[:, C_syn + ST + fo - caps_sem[e] : C_syn + ST + fo, :])

    nc.compile()
    return nc


def _get_program(geom):
    if geom not in _prog_cache:
        _prog_cache[geom] = _build_program(geom)
    return _prog_cache[geom]


# ---------------------------------------------------------------- host glue
def _prepare(inputs):
    """Compute routing, permutations, windows; build per-core in_maps and
    decode metadata. Returns (geom, in_maps, meta)."""
    hs = np.asarray(inputs["hidden_states"], dtype=np.float32)
    adj = np.asarray(inputs["adj_matrix"], dtype=np.float32)
    seq_lengths = np.asarray(inputs["seq_lengths"])
    router_w = np.asarray(inputs["router_w"], dtype=np.float32)
    router_b = np.asarray(inputs["router_b"], dtype=np.float32)
    gcn1_w = np.asarray(inputs["gcn1_w"], dtype=np.float32)
    gcn2_w = np.asarray(inputs["gcn2_w"], dtype=np.float32)
    ln_g = np.asarray(inputs["ln_g"], dtype=np.float32)
    ln_b = np.asarray(inputs["ln_b"], dtype=np.float32)
    syn_w = np.asarray(inputs["syn_w"], dtype=np.float32)
    syn_b = np.asarray(inputs["syn_b"], dtype=np.float32)
    len_short_w = np.asarray(inputs["len_short_w"], dtype=np.float32)
    len_short_b = np.asarray(inputs["len_short_b"], dtype=np.float32)
    len_long_w = np.asarray(inputs["len_long_w"], dtype=np.float32)
    len_long_b = np.asarray(inputs["len_long_b"], dtype=np.float32)
    sem_w = np.asarray(inputs["sem_w"], dtype=np.float32)
    sem_b = np.asarray(inputs["sem_b"], dtype=np.float32)
    cls_w = np.asarray(inputs["cls_w"], dtype=np.float32)
    cls_b = np.asarray(inputs["cls_b"], dtype=np.float32)

    c_syn, syn_i, c_len, c_sem, sem_i, is_short = _route_host(
        hs, router_w, router_b, seq_lengths)

    # fold LN affine into syn weights: LN_plain(x) @ (g*W) + (b@W + bias)
    syn_w_f = (ln_g[None, :, None] * syn_w).astype(np.float32)
    syn_b_f = (syn_b + np.einsum("h,ehd->ed", ln_b, syn_w)).astype(np.float32)

    perm = np.argsort(syn_i, axis=1, kind="stable")          # [B,S]
    syn_i_p = np.take_along_axis(syn_i, perm, axis=1)
    sem_i_p = np.take_along_axis(sem_i, perm, axis=1)
    sem_perm = np.argsort(sem_i_p, axis=1, kind="stable")    # syn-order -> sem-order
    sem_i_s = np.take_along_axis(sem_i_p, sem_perm, axis=1)

    re_syn, caps_syn = _windows(syn_i_p)
    re_sem, caps_sem = _windows(sem_i_s)
    C_syn, C_sem = sum(caps_syn), sum(caps_sem)

    synb_nz = bool(np.any(syn_b_f != 0))
    lenb_nz = bool(np.any(len_short_b != 0) or np.any(len_long_b != 0))
    semb_nz = bool(np.any(sem_b != 0))
    geom = (re_syn, caps_syn, re_sem, caps_sem, synb_nz, lenb_nz, semb_nz)

    wg1 = np.ascontiguousarray(gcn1_w.astype(_BF16))
    wg2 = np.ascontiguousarray(gcn2_w.astype(_BF16))
    wsyn = np.ascontiguousarray(syn_w_f.astype(_BF16))
    wlen_s = np.ascontiguousarray(len_short_w.astype(_BF16))
    wlen_l = np.ascontiguousarray(len_long_w.astype(_BF16))
    wsem = np.ascontiguousarray(sem_w.astype(_BF16))
    wcls = np.ascontiguousarray(cls_w.astype(_BF16))

    def win_coef(cvec, idx_sorted_row, re, caps, grp):
        """Per-window masked coefficients, zero-padded to caps*P."""
        out = np.zeros(max(sum(caps), 1) * P, np.float32)
        off = 0
        for e in range(3):
            w = caps[e] * P
            lo = re[e]
            hi = min(S, lo + w)
            seg = np.where(idx_sorted_row[lo:hi] == e, cvec[lo:hi], 0.0)
            out[off : off + (hi - lo)] = seg
            off += w
        return out

    in_maps = []
    meta = []
    for b in range(B):
        p = perm[b]
        sp = sem_perm[b]
        hs_p = hs[b][p]
        adj_p = adj[b][p][:, p]
        deg = np.maximum(adj_p.sum(axis=1, dtype=np.float32), 1e-9)
        adj_n = adj_p / deg[:, None]
        hs_sem = hs_p[sp]
        c_syn_p = c_syn[b][p]
        c_len_p = c_len[b][p]
        c_sem_s = c_sem[b][p][sp]

        m = {
            "hsT": np.ascontiguousarray(hs_p.T.astype(_BF16)),
            "adjT": np.ascontiguousarray(adj_n.T.astype(_BF16)),
            "hs": np.ascontiguousarray(hs_p),
            "semT": np.ascontiguousarray(hs_sem.T.astype(_BF16)),
            "wg1": wg1, "wg2": wg2, "wsyn": wsyn,
            "wlen": wlen_s if is_short[b] else wlen_l,
            "wsem": wsem, "wcls": wcls,
            "csyn": win_coef(c_syn_p, syn_i_p[b], re_syn, caps_syn, "syn"),
            "clen": np.ascontiguousarray(c_len_p),
            "csem": win_coef(c_sem_s, sem_i_s[b], re_sem, caps_sem, "sem"),
        }
        if synb_nz:
            m["bsyn"] = syn_b_f
        if lenb_nz:
            m["blen"] = (len_short_b if is_short[b]
                         else len_long_b).reshape(1, H).astype(np.float32)
        if semb_nz:
            m["bsem"] = sem_b.astype(np.float32)
        in_maps.append(m)
        meta.append((p, sp))

    return geom, in_maps, meta, cls_b


def _decode(out_rows, geom, meta_b, cls_b):
    """out_rows: [NT*P, 2] device output for one core -> [S,2] original order."""
    re_syn, caps_syn, re_sem, caps_sem = geom[0], geom[1], geom[2], geom[3]
    C_syn, C_sem = sum(caps_syn), sum(caps_sem)
    p, sp = meta_b
    acc_syn = np.zeros((S, 2), np.float32)   # syn-order accumulation
    off = 0
    for e in range(3):
        w = caps_syn[e] * P
        lo = re_syn[e]
        hi = min(S, lo + w)
        acc_syn[lo:hi] += out_rows[off : off + (hi - lo)]
        off += w
    acc_syn += out_rows[C_syn * P : C_syn * P + S]          # len group
    acc_sem = np.zeros((S, 2), np.float32)   # sem-order
    off = (C_syn + ST) * P
    for e in range(3):
        w = caps_sem[e] * P
        lo = re_sem[e]
        hi = min(S, lo + w)
        acc_sem[lo:hi] += out_rows[off : off + (hi - lo)]
        off += w
    acc_syn[sp] += acc_sem
    res = np.empty((S, 2), np.float32)
    res[p] = acc_syn
    return res + cls_b


def kernel(**inputs):
    from concourse import bass_utils

    geom, in_maps, meta, cls_b = _prepare(inputs)
    nc = _get_program(geom)

    try:
        res = bass_utils.run_bass_kernel_spmd(nc, in_maps, core_ids=list(range(B)))
    except Exception:
        # transient device wedge (NRT_EXEC_UNIT_UNRECOVERABLE) clears on retry
        res = bass_utils.run_bass_kernel_spmd(nc, in_maps, core_ids=list(range(B)))
    globals()["_last_results"] = res
    out = np.stack([_decode(np.asarray(res.results[b]["out"], np.float32),
                            geom, meta[b], cls_b)
                    for b in range(B)]).astype(np.float32)
    return out
